# revision 14
# baseline (speedup 1.0000x reference)
"""Trainium2 Bass kernel for Cell2Vec GNN message passing (8 NeuronCores).

Math: 3x GraphConv (DGL norm='both') + node-select + projection + cell-embedding
scores:
    out = emb[c_indices] @ (relu-chain...)  -> [N_C, N_SEL]

Restructure used on device (per layer):
    H_next = relu( Ahat @ (H @ W) + b ),  Ahat = D_in^-1/2 A D_out^-1/2
with the degree norms folded into per-edge weights w_e = ns[src] * nd[dst].

Sharding: nodes are dst-sharded across 8 cores (6250 each, padded 6272 = 49
tiles of 128). Per layer, each core computes Z = H_own @ W (dense, PE), an
AllGather replicates Z to all cores, then each core aggregates its owned
dst-nodes: for each dst-bin (128 nodes) and edge-tile (128 edges), gather the
128 src rows of Z (indirect DMA) and accumulate on the tensor engine
    aggT[feat, dstslot] += msg[lane, feat]^T @ Onehot[lane, dstslot]
where Onehot carries w_e at (lane, dst_slot). This yields H_next^T directly
(feature-major), which is exactly the lhsT layout the next dense needs.
Layer 3 only aggregates into the x_indices-selected nodes. The final
projection + emb @ proj^T runs per-core on owned selected columns; the host
reassembles the [1024, 8192] output from per-core column blocks.

Bins are in-degree balanced per core (host preprocessing) so every bin has
the same number of edge tiles K; all 8 cores run one identical SPMD program.
"""
import heapq
import numpy as np
import ml_dtypes

P = 128
C = 8

# full-problem config (hardcoded per spec; kernel.py must be self-contained)
N_NODES = 50000
N_EDGES = 400000
IN_F = 512
HID = 512
OUT_F = 256
N_CELL = 1000
N_DIM = 128
N_SEL = 8192
N_C = 1024

BF16 = ml_dtypes.bfloat16

_COMPILE_CACHE = {}
LAST_EXEC_TIME_NS = None
TRACE = False


# ----------------------------------------------------------------------------
# host preprocessing
# ----------------------------------------------------------------------------

def _balance_bins(weights, n_bins, cap):
    """Greedy balanced binning: heaviest first into least-loaded open bin.
    Returns (bin_of_item, slot_of_item)."""
    order = np.argsort(-weights, kind="stable")
    heap = [(0.0, b) for b in range(n_bins)]
    heapq.heapify(heap)
    counts = np.zeros(n_bins, np.int64)
    bin_of = np.empty(len(weights), np.int64)
    slot_of = np.empty(len(weights), np.int64)
    for i in order:
        spill = []
        while True:
            load, b = heapq.heappop(heap)
            if counts[b] < cap:
                break
            spill.append((load, b))
        bin_of[i] = b
        slot_of[i] = counts[b]
        counts[b] += 1
        heapq.heappush(heap, (load + float(weights[i]), b))
        for s in spill:
            heapq.heappush(heap, s)
    return bin_of, slot_of


def _group_edges(key, n_groups, payload_order):
    """Sort edges by group key; return per-group start/end and sorted order."""
    order = np.argsort(key, kind="stable")
    ks = key[order]
    gs = np.searchsorted(ks, np.arange(n_groups))
    ge = np.searchsorted(ks, np.arange(n_groups), side="right")
    return order, ks, gs, ge


def preprocess(x, src, dst, x_indices, c_indices):
    src = np.asarray(src).astype(np.int64)
    dst = np.asarray(dst).astype(np.int64)
    x_indices = np.asarray(x_indices).astype(np.int64)
    c_indices = np.asarray(c_indices).astype(np.int64)
    x = np.asarray(x)
    n = x.shape[0]
    nshard = n // C
    nt = (nshard + P - 1) // P
    npad = nt * P

    deg_out = np.bincount(src, minlength=n).astype(np.float64)
    deg_in = np.bincount(dst, minlength=n).astype(np.float64)
    ns = np.where(deg_out > 0, 1.0 / np.sqrt(np.maximum(deg_out, 1.0)), 0.0)
    nd = np.where(deg_in > 0, 1.0 / np.sqrt(np.maximum(deg_in, 1.0)), 0.0)
    w_e = (ns[src] * nd[dst]).astype(np.float32)

    owner_n = np.arange(n) // nshard
    localrow = np.empty(n, np.int64)
    for c in range(C):
        nodes = np.arange(c * nshard, (c + 1) * nshard)
        b, s = _balance_bins(deg_in[nodes], nt, P)
        localrow[nodes] = b * P + s
    prow = owner_n * npad + localrow

    # L1/L2 edge layout
    owner_e = dst // nshard
    key = owner_e * nt + localrow[dst] // P
    order, ks, gs, ge = _group_edges(key, C * nt, None)
    K = int(np.ceil((ge - gs).max() / P))
    pos = np.arange(len(src)) - gs[ks]
    cc, bb = ks // nt, ks % nt
    kk, pp = pos // P, pos % P
    es = order
    gidx = np.zeros((C, nt, K, P), np.int32)
    ohw = np.zeros((C, nt, K, P), np.float32)
    ohq = np.zeros((C, nt, K, P), np.int64)
    gidx[cc, bb, kk, pp] = prow[src[es]].astype(np.int32)
    ohw[cc, bb, kk, pp] = w_e[es]
    ohq[cc, bb, kk, pp] = localrow[dst[es]] % P

    # L3: selected nodes only
    sel_nodes = np.unique(x_indices)
    sel_mask = np.zeros(n, bool)
    sel_mask[sel_nodes] = True
    e3 = np.nonzero(sel_mask[dst])[0]
    deg3 = np.bincount(dst[e3], minlength=n).astype(np.float64)
    ncol_max = max(int((sel_nodes // nshard == c).sum()) for c in range(C))
    T3 = max(1, (ncol_max + P - 1) // P)
    ncol = T3 * P
    colpos = np.full(n, 0, np.int64)
    selrow_cols = np.zeros((C, ncol), np.int32)   # col -> local row (for enc gather)
    for c in range(C):
        nodes = sel_nodes[sel_nodes // nshard == c]
        b, s = _balance_bins(deg3[nodes], T3, P)
        colpos[nodes] = b * P + s
        selrow_cols[c, b * P + s] = localrow[nodes].astype(np.int32)
    d3, s3 = dst[e3], src[e3]
    key3 = (d3 // nshard) * T3 + colpos[d3] // P
    order3, ks3, g3s, g3e = _group_edges(key3, C * T3, None)
    K3 = max(1, int(np.ceil((g3e - g3s).max() / P)))
    pos3 = np.arange(len(e3)) - g3s[ks3]
    cc3, bb3 = ks3 // T3, ks3 % T3
    kk3, pp3 = pos3 // P, pos3 % P
    es3 = e3[order3]
    gidx3 = np.zeros((C, T3, K3, P), np.int32)
    ohw3 = np.zeros((C, T3, K3, P), np.float32)
    ohq3 = np.zeros((C, T3, K3, P), np.int64)
    gidx3[cc3, bb3, kk3, pp3] = prow[src[es3]].astype(np.int32)
    ohw3[cc3, bb3, kk3, pp3] = w_e[es3]
    ohq3[cc3, bb3, kk3, pp3] = colpos[dst[es3]] % P

    xi_owner = (x_indices // nshard).astype(np.int32)
    xi_col = colpos[x_indices].astype(np.int32)

    # dense one-hot tiles, device layout [bin, lane, k*P+q]
    def onehots(w, q, ntiles, Kt):
        out = np.zeros((C, ntiles, P, Kt * P), BF16)
        ci, di, ki, pi = np.indices(w.shape)
        out[ci, di, pi, ki * P + q] = w.astype(BF16)
        return out

    oh12 = onehots(ohw, ohq, nt, K)
    oh3 = onehots(ohw3, ohq3, T3, K3)

    # gather index tables, device layout [lane, bin*K + k]
    gidxT = np.ascontiguousarray(gidx.transpose(0, 3, 1, 2).reshape(C, P, nt * K))
    gidxT3 = np.ascontiguousarray(gidx3.transpose(0, 3, 1, 2).reshape(C, P, T3 * K3))

    # per-core permuted x^T in [128, 4, npad] chunk layout
    F = x.shape[1]
    FC = F // P
    xT = np.zeros((C, P, FC, npad), BF16)
    for c in range(C):
        nodes = np.arange(c * nshard, (c + 1) * nshard)
        xv = x[nodes].astype(BF16)            # [nshard, F]
        for fc in range(FC):
            xT[c, :, fc, localrow[nodes]] = xv[:, fc * P:(fc + 1) * P]
    xT = xT.reshape(C, P, FC * npad)

    return dict(
        n=n, nshard=nshard, nt=nt, npad=npad, K=K, K3=K3, T3=T3, ncol=ncol,
        gidxT=gidxT, gidxT3=gidxT3, oh12=oh12, oh3=oh3, xT=xT,
        selrow_cols=selrow_cols, xi_owner=xi_owner, xi_col=xi_col,
    )


def _pack_weights(W1, b1, W2, b2, W3, b3, Wp, bp, emb, c_indices):
    """Device layouts: W [fin, fout] -> [128, nchunk*fout]; b -> [128, nchunk]."""
    def wdev(W):
        fin, fout = W.shape
        nc_ = fin // P
        return np.ascontiguousarray(
            W.astype(BF16).reshape(nc_, P, fout).transpose(1, 0, 2).reshape(P, nc_ * fout))

    def bdev(b):
        nc_ = len(b) // P
        return np.ascontiguousarray(
            np.asarray(b, np.float32).reshape(nc_, P).T)

    c_idx = np.asarray(c_indices, np.int64)
    ncg = (len(c_idx) + P - 1) // P
    tmp = np.zeros(ncg * P, np.int32)
    tmp[:len(c_idx)] = c_idx
    cidx_dev = np.ascontiguousarray(tmp.reshape(ncg, P).T)
    return dict(
        W1=wdev(W1), W2=wdev(W2), W3=wdev(W3), Wp=wdev(Wp),
        b1=bdev(b1), b2=bdev(b2), b3=bdev(b3), bp=bdev(bp),
        emb=np.asarray(emb, np.float32), cidx=cidx_dev, ncg=ncg,
    )


# ----------------------------------------------------------------------------
# bass program
# ----------------------------------------------------------------------------

def build_program(meta):
    import concourse.bacc as bacc
    import concourse.bass as bass
    import concourse.mybir as mybir
    import concourse.tile as tile
    from concourse.masks import make_identity

    nt, npad, K = meta["nt"], meta["npad"], meta["K"]
    K3, T3, ncol = meta["K3"], meta["T3"], meta["ncol"]
    ncg = meta["ncg"]
    hid, out_f = meta["hid"], meta["out_f"]
    n_cell, n_dim, n_c = meta["n_cell"], meta["n_dim"], meta["n_c"]
    nrows = C * npad
    FCH = hid // P            # chunks of hidden width
    FCO = out_f // P          # chunks of layer-3 output width
    dt = mybir.dt
    AF = mybir.ActivationFunctionType

    nc = bacc.Bacc("TRN2", target_bir_lowering=False, debug=False, num_devices=C)

    def din(name, shape, dtype):
        return nc.dram_tensor(name, list(shape), dtype, kind="ExternalInput").ap()

    xT_d = din("xT", (P, FCH * npad), dt.bfloat16)
    oh12_d = din("oh12", (nt, P, K * P), dt.bfloat16)
    oh3_d = din("oh3", (T3, P, K3 * P), dt.bfloat16)
    gidxT_d = din("gidxT", (P, nt * K), dt.int32)
    gidxT3_d = din("gidxT3", (P, T3 * K3), dt.int32)
    cidx_d = din("cidx", (P, ncg), dt.int32)
    W1_d = din("W1", (P, FCH * hid), dt.bfloat16)
    W2_d = din("W2", (P, FCH * hid), dt.bfloat16)
    W3_d = din("W3", (P, FCH * out_f), dt.bfloat16)
    Wp_d = din("Wp", (P, FCO * n_dim), dt.bfloat16)
    b1_d = din("b1", (P, FCH), dt.float32)
    b2_d = din("b2", (P, FCH), dt.float32)
    b3_d = din("b3", (P, FCO), dt.float32)
    bp_d = din("bp", (P, 1), dt.float32)
    emb_d = din("emb", (n_cell, n_dim), dt.float32)

    out_d = nc.dram_tensor("outc", [n_c, ncol], dt.float32, kind="ExternalOutput").ap()
    if meta.get("debug"):
        dbg_h3 = nc.dram_tensor("dbg_h3", [P, (out_f // P) * ncol], dt.bfloat16,
                                kind="ExternalOutput").ap()
        dbg_proj = nc.dram_tensor("dbg_proj", [P, ncol], dt.bfloat16,
                                  kind="ExternalOutput").ap()
        dbg_emb = nc.dram_tensor("dbg_emb", [P, ncg * P], dt.bfloat16,
                                 kind="ExternalOutput").ap()

    zfull = [
        nc.dram_tensor(f"zfull{i}", [nrows, hid if i < 2 else out_f], dt.bfloat16,
                       kind="Internal", addr_space="Shared").ap()
        for i in range(3)
    ]

    with tile.TileContext(nc) as tc:
        with tc.tile_pool(name="dram", bufs=1, space="DRAM") as dram, \
             tc.tile_pool(name="persist", bufs=1) as persist, \
             tc.tile_pool(name="wpool", bufs=1) as wpool, \
             tc.tile_pool(name="sbuf", bufs=3) as sbuf, \
             tc.tile_pool(name="msgp", bufs=2 * max(K, K3) + 2) as msgp, \
             tc.tile_pool(name="ohp", bufs=3) as ohp, \
             tc.tile_pool(name="zst", bufs=3) as zst, \
             tc.tile_pool(name="psum_d", bufs=2, space="PSUM") as psum_d, \
             tc.tile_pool(name="psum_a", bufs=4, space="PSUM") as psum_a:

            # persistent tiles
            HT_a = persist.tile([P, FCH * npad], dt.bfloat16, tag="HT_a")
            HT_b = persist.tile([P, FCH * npad], dt.bfloat16, tag="HT_b")
            H3T = persist.tile([P, FCO * ncol], dt.bfloat16, tag="H3T")
            gidxT_t = persist.tile([P, nt * K], dt.int32, tag="gidx")
            gidxT3_t = persist.tile([P, T3 * K3], dt.int32, tag="gidx3")
            ident = persist.tile([P, P], dt.float32, tag="ident")
            make_identity(nc, ident[:])

            nc.sync.dma_start(HT_a[:], xT_d[:])
            nc.sync.dma_start(gidxT_t[:], gidxT_d[:])
            nc.sync.dma_start(gidxT3_t[:], gidxT3_d[:])

            def dense(HT, W_ap, fout, zf_idx):
                """Z = H_own @ W -> DRAM (bf16, node-major), then AllGather."""
                Wt = wpool.tile([P, FCH * fout], dt.bfloat16, tag="W")
                nc.sync.dma_start(Wt[:], W_ap[:])
                zc = dram.tile([npad, fout], dt.bfloat16, tag=f"zc{zf_idx}")
                for i in range(nt):
                    ps = psum_d.tile([P, fout], dt.float32, space="PSUM", tag="pd")
                    for f in range(FCH):
                        nc.tensor.matmul(
                            ps[:],
                            lhsT=HT[:, f * npad + i * P: f * npad + (i + 1) * P],
                            rhs=Wt[:, f * fout:(f + 1) * fout],
                            start=(f == 0), stop=(f == FCH - 1))
                    zs = zst.tile([P, fout], dt.bfloat16, tag="zs")
                    nc.vector.tensor_copy(zs[:], ps[:])
                    nc.sync.dma_start(zc[i * P:(i + 1) * P, :], zs[:])
                nc.gpsimd.collective_compute(
                    "AllGather", mybir.AluOpType.bypass,
                    replica_groups=[list(range(C))],
                    ins=[zc[:]], outs=[zfull[zf_idx]])

            def aggregate(zf_idx, oh_ap, gidx_t, b_ap, HTout, ntiles, Kt, fch):
                """H_out^T[:, bin] = relu( sum_k msg_k^T @ oh_k + b )."""
                bt = wpool.tile([P, fch], dt.float32, tag="b")
                nc.sync.dma_start(bt[:], b_ap[:])
                zf = zfull[zf_idx]
                for d in range(ntiles):
                    oh_t = ohp.tile([P, Kt * P], dt.bfloat16, tag="oh")
                    nc.sync.dma_start(oh_t[:], oh_ap[d])
                    ps = psum_a.tile([P, fch * P], dt.float32, space="PSUM", tag="pa")
                    msgs = []
                    for k in range(Kt):
                        msg = msgp.tile([P, fch * P], dt.bfloat16, tag="msg")
                        nc.gpsimd.indirect_dma_start(
                            out=msg[:], out_offset=None, in_=zf,
                            in_offset=bass.IndirectOffsetOnAxis(
                                ap=gidx_t[:, d * Kt + k: d * Kt + k + 1], axis=0))
                        msgs.append(msg)
                    for f in range(fch):
                        for k in range(Kt):
                            nc.tensor.matmul(
                                ps[:, f * P:(f + 1) * P],
                                lhsT=msgs[k][:, f * P:(f + 1) * P],
                                rhs=oh_t[:, k * P:(k + 1) * P],
                                start=(k == 0), stop=(k == Kt - 1))
                    for f in range(fch):
                        nc.scalar.activation(
                            HTout[:, f * (ntiles * P) + d * P:
                                  f * (ntiles * P) + (d + 1) * P],
                            ps[:, f * P:(f + 1) * P],
                            AF.Relu, bias=bt[:, f:f + 1])

            # ---- layers ----
            dense(HT_a, W1_d, hid, 0)
            aggregate(0, oh12_d, gidxT_t, b1_d, HT_b, nt, K, FCH)
            dense(HT_b, W2_d, hid, 1)
            aggregate(1, oh12_d, gidxT_t, b2_d, HT_a, nt, K, FCH)
            dense(HT_a, W3_d, out_f, 2)
            aggregate(2, oh3_d, gidxT3_t, b3_d, H3T, T3, K3, FCO)

            # ---- projection: projT = Wp^T @ enc^T + bp  [n_dim, ncol] ----
            Wpt = wpool.tile([P, FCO * n_dim], dt.bfloat16, tag="W")
            bpt = wpool.tile([P, 1], dt.float32, tag="b")
            nc.sync.dma_start(Wpt[:], Wp_d[:])
            nc.sync.dma_start(bpt[:], bp_d[:])
            projT = persist.tile([P, ncol], dt.bfloat16, tag="projT")
            nseg = (ncol + 511) // 512
            for s in range(nseg):
                w = min(512, ncol - s * 512)
                pp = psum_d.tile([P, 512], dt.float32, space="PSUM", tag="pd")
                for f in range(FCO):
                    nc.tensor.matmul(
                        pp[:, :w],
                        lhsT=Wpt[:, f * n_dim:(f + 1) * n_dim],
                        rhs=H3T[:, f * ncol + s * 512: f * ncol + s * 512 + w],
                        start=(f == 0), stop=(f == FCO - 1))
                nc.scalar.activation(projT[:, s * 512:s * 512 + w], pp[:, :w],
                                     AF.Identity, bias=bpt[:, 0:1])

            # ---- EmbSel^T: gather emb[c_indices] and transpose ----
            cidx_t = sbuf.tile([P, ncg], dt.int32, tag="cidx")
            nc.sync.dma_start(cidx_t[:], cidx_d[:])
            embT = persist.tile([P, ncg * P], dt.bfloat16, tag="embT")
            for g in range(ncg):
                e_g = sbuf.tile([P, n_dim], dt.float32, tag="eg")
                nc.gpsimd.indirect_dma_start(
                    out=e_g[:], out_offset=None, in_=emb_d,
                    in_offset=bass.IndirectOffsetOnAxis(
                        ap=cidx_t[:, g:g + 1], axis=0))
                pt = psum_d.tile([P, P], dt.float32, space="PSUM", tag="pd")
                nc.tensor.transpose(pt[:], e_g[:], ident[:])
                nc.vector.tensor_copy(embT[:, g * P:(g + 1) * P], pt[:])

            if meta.get("debug"):
                nc.sync.dma_start(dbg_h3[:], H3T[:])
                nc.sync.dma_start(dbg_proj[:], projT[:])
                nc.sync.dma_start(dbg_emb[:], embT[:])

            # ---- out_c = EmbSel @ projT  [N_C, ncol] ----
            for g in range(ncg):
                for s in range(nseg):
                    w = min(512, ncol - s * 512)
                    po = psum_d.tile([P, 512], dt.float32, space="PSUM", tag="pd")
                    nc.tensor.matmul(
                        po[:, :w],
                        lhsT=embT[:, g * P:(g + 1) * P],
                        rhs=projT[:, s * 512:s * 512 + w],
                        start=True, stop=True)
                    os_ = zst.tile([P, 512], dt.float32, tag="os")
                    nc.vector.tensor_copy(os_[:, :w], po[:, :w])
                    nc.sync.dma_start(
                        out_d[g * P:(g + 1) * P, s * 512:s * 512 + w],
                        os_[:, :w])

    nc.compile()
    return nc


# ----------------------------------------------------------------------------
# entry point
# ----------------------------------------------------------------------------

def _ensure_ntff_hook():
    """Register the axon NTFF-profile hook if the image's antenv lacks it.
    Only used on the TRACE path (benchmarking); grading runs trace=False."""
    import sys
    import types
    try:
        from antenv.axon_hooks import get_axon_ntff_profile_hook  # noqa: F401
        return
    except ImportError:
        pass
    try:
        from trn_agent_boot.trn_boot import _ntff_profile_via_ctypes
        hook = _ntff_profile_via_ctypes("/opt/axon/libaxon_pjrt.so")
    except Exception:
        hook = None
    mod = types.ModuleType("antenv.axon_hooks")
    mod._hook = hook
    mod.get_axon_ntff_profile_hook = lambda: mod._hook
    mod.set_axon_ntff_profile_hook = lambda h: setattr(mod, "_hook", h)
    import antenv
    antenv.axon_hooks = mod
    sys.modules["antenv.axon_hooks"] = mod


def kernel(**inputs):
    global LAST_EXEC_TIME_NS
    from concourse import bass_utils
    if TRACE:
        _ensure_ntff_hook()

    x = np.asarray(inputs["x"], np.float32)
    prep = preprocess(x, inputs["src"], inputs["dst"],
                      inputs["x_indices"], inputs["c_indices"])
    wp = _pack_weights(inputs["W1"], inputs["b1"], inputs["W2"], inputs["b2"],
                       inputs["W3"], inputs["b3"], inputs["Wp"], inputs["bp"],
                       inputs["emb"], inputs["c_indices"])

    hid = np.asarray(inputs["W1"]).shape[1]
    out_f = np.asarray(inputs["W3"]).shape[1]
    n_dim = np.asarray(inputs["Wp"]).shape[1]
    n_cell = np.asarray(inputs["emb"]).shape[0]
    n_c = len(np.asarray(inputs["c_indices"]))
    meta = dict(nt=prep["nt"], npad=prep["npad"], K=prep["K"], K3=prep["K3"],
                T3=prep["T3"], ncol=prep["ncol"], ncg=wp["ncg"],
                hid=hid, out_f=out_f, n_dim=n_dim, n_cell=n_cell, n_c=n_c)
    meta_key = tuple(sorted(meta.items()))
    if meta_key not in _COMPILE_CACHE:
        _COMPILE_CACHE[meta_key] = build_program(meta)
    nc = _COMPILE_CACHE[meta_key]

    in_maps = []
    for c in range(C):
        in_maps.append({
            "xT": prep["xT"][c],
            "oh12": prep["oh12"][c],
            "oh3": prep["oh3"][c],
            "gidxT": prep["gidxT"][c],
            "gidxT3": prep["gidxT3"][c],
            "cidx": wp["cidx"],
            "W1": wp["W1"], "W2": wp["W2"], "W3": wp["W3"], "Wp": wp["Wp"],
            "b1": wp["b1"], "b2": wp["b2"], "b3": wp["b3"], "bp": wp["bp"],
            "emb": wp["emb"],
        })

    res = bass_utils.run_bass_kernel_spmd(
        nc, in_maps, core_ids=list(range(C)), trace=TRACE)
    LAST_EXEC_TIME_NS = res.exec_time_ns

    outs = np.stack([r["outc"] for r in res.results])     # [C, N_C, ncol]
    final = outs[prep["xi_owner"], :, prep["xi_col"]]     # [N_SEL, N_C]
    return np.ascontiguousarray(final.T, np.float32)      # [N_C, N_SEL]


# revision 15
# speedup vs baseline: 1.0034x; 1.0034x over previous
"""Trainium2 Bass kernel for Cell2Vec GNN message passing (8 NeuronCores).

Math: 3x GraphConv (DGL norm='both') + node-select + projection + cell-embedding
scores:
    out = emb[c_indices] @ (relu-chain...)  -> [N_C, N_SEL]

Restructure used on device (per layer):
    H_next = relu( Ahat @ (H @ W) + b ),  Ahat = D_in^-1/2 A D_out^-1/2
with the degree norms folded into per-edge weights w_e = ns[src] * nd[dst].

Sharding: nodes are dst-sharded across 8 cores (6250 each, padded 6272 = 49
tiles of 128). Per layer, each core computes Z = H_own @ W (dense, PE), an
AllGather replicates Z to all cores, then each core aggregates its owned
dst-nodes: for each dst-bin (128 nodes) and edge-tile (128 edges), gather the
128 src rows of Z (indirect DMA) and accumulate on the tensor engine
    aggT[feat, dstslot] += msg[lane, feat]^T @ Onehot[lane, dstslot]
where Onehot carries w_e at (lane, dst_slot). This yields H_next^T directly
(feature-major), which is exactly the lhsT layout the next dense needs.
Layer 3 only aggregates into the x_indices-selected nodes. The final
projection + emb @ proj^T runs per-core on owned selected columns; the host
reassembles the [1024, 8192] output from per-core column blocks.

Bins are in-degree balanced per core (host preprocessing) so every bin has
the same number of edge tiles K; all 8 cores run one identical SPMD program.
"""
import heapq
import numpy as np
import ml_dtypes

P = 128
C = 8

# full-problem config (hardcoded per spec; kernel.py must be self-contained)
N_NODES = 50000
N_EDGES = 400000
IN_F = 512
HID = 512
OUT_F = 256
N_CELL = 1000
N_DIM = 128
N_SEL = 8192
N_C = 1024

BF16 = ml_dtypes.bfloat16

_COMPILE_CACHE = {}
LAST_EXEC_TIME_NS = None
TRACE = False


# ----------------------------------------------------------------------------
# host preprocessing
# ----------------------------------------------------------------------------

def _balance_bins(weights, n_bins, cap):
    """Greedy balanced binning: heaviest first into least-loaded open bin.
    Returns (bin_of_item, slot_of_item)."""
    order = np.argsort(-weights, kind="stable")
    heap = [(0.0, b) for b in range(n_bins)]
    heapq.heapify(heap)
    counts = np.zeros(n_bins, np.int64)
    bin_of = np.empty(len(weights), np.int64)
    slot_of = np.empty(len(weights), np.int64)
    for i in order:
        spill = []
        while True:
            load, b = heapq.heappop(heap)
            if counts[b] < cap:
                break
            spill.append((load, b))
        bin_of[i] = b
        slot_of[i] = counts[b]
        counts[b] += 1
        heapq.heappush(heap, (load + float(weights[i]), b))
        for s in spill:
            heapq.heappush(heap, s)
    return bin_of, slot_of


def _group_edges(key, n_groups, payload_order):
    """Sort edges by group key; return per-group start/end and sorted order."""
    order = np.argsort(key, kind="stable")
    ks = key[order]
    gs = np.searchsorted(ks, np.arange(n_groups))
    ge = np.searchsorted(ks, np.arange(n_groups), side="right")
    return order, ks, gs, ge


def preprocess(x, src, dst, x_indices, c_indices):
    src = np.asarray(src).astype(np.int64)
    dst = np.asarray(dst).astype(np.int64)
    x_indices = np.asarray(x_indices).astype(np.int64)
    c_indices = np.asarray(c_indices).astype(np.int64)
    x = np.asarray(x)
    n = x.shape[0]
    nshard = n // C
    nt = (nshard + P - 1) // P
    npad = nt * P

    deg_out = np.bincount(src, minlength=n).astype(np.float64)
    deg_in = np.bincount(dst, minlength=n).astype(np.float64)
    ns = np.where(deg_out > 0, 1.0 / np.sqrt(np.maximum(deg_out, 1.0)), 0.0)
    nd = np.where(deg_in > 0, 1.0 / np.sqrt(np.maximum(deg_in, 1.0)), 0.0)
    w_e = (ns[src] * nd[dst]).astype(np.float32)

    owner_n = np.arange(n) // nshard
    localrow = np.empty(n, np.int64)
    for c in range(C):
        nodes = np.arange(c * nshard, (c + 1) * nshard)
        b, s = _balance_bins(deg_in[nodes], nt, P)
        localrow[nodes] = b * P + s
    prow = owner_n * npad + localrow

    # L1/L2 edge layout
    owner_e = dst // nshard
    key = owner_e * nt + localrow[dst] // P
    order, ks, gs, ge = _group_edges(key, C * nt, None)
    K = int(np.ceil((ge - gs).max() / P))
    pos = np.arange(len(src)) - gs[ks]
    cc, bb = ks // nt, ks % nt
    kk, pp = pos // P, pos % P
    es = order
    gidx = np.zeros((C, nt, K, P), np.int32)
    ohw = np.zeros((C, nt, K, P), np.float32)
    ohq = np.zeros((C, nt, K, P), np.int64)
    gidx[cc, bb, kk, pp] = prow[src[es]].astype(np.int32)
    ohw[cc, bb, kk, pp] = w_e[es]
    ohq[cc, bb, kk, pp] = localrow[dst[es]] % P

    # L3: selected nodes only
    sel_nodes = np.unique(x_indices)
    sel_mask = np.zeros(n, bool)
    sel_mask[sel_nodes] = True
    e3 = np.nonzero(sel_mask[dst])[0]
    deg3 = np.bincount(dst[e3], minlength=n).astype(np.float64)
    ncol_max = max(int((sel_nodes // nshard == c).sum()) for c in range(C))
    T3 = max(1, (ncol_max + P - 1) // P)
    ncol = T3 * P
    colpos = np.full(n, 0, np.int64)
    selrow_cols = np.zeros((C, ncol), np.int32)   # col -> local row (for enc gather)
    for c in range(C):
        nodes = sel_nodes[sel_nodes // nshard == c]
        b, s = _balance_bins(deg3[nodes], T3, P)
        colpos[nodes] = b * P + s
        selrow_cols[c, b * P + s] = localrow[nodes].astype(np.int32)
    d3, s3 = dst[e3], src[e3]
    key3 = (d3 // nshard) * T3 + colpos[d3] // P
    order3, ks3, g3s, g3e = _group_edges(key3, C * T3, None)
    K3 = max(1, int(np.ceil((g3e - g3s).max() / P)))
    pos3 = np.arange(len(e3)) - g3s[ks3]
    cc3, bb3 = ks3 // T3, ks3 % T3
    kk3, pp3 = pos3 // P, pos3 % P
    es3 = e3[order3]
    gidx3 = np.zeros((C, T3, K3, P), np.int32)
    ohw3 = np.zeros((C, T3, K3, P), np.float32)
    ohq3 = np.zeros((C, T3, K3, P), np.int64)
    gidx3[cc3, bb3, kk3, pp3] = prow[src[es3]].astype(np.int32)
    ohw3[cc3, bb3, kk3, pp3] = w_e[es3]
    ohq3[cc3, bb3, kk3, pp3] = colpos[dst[es3]] % P

    xi_owner = (x_indices // nshard).astype(np.int32)
    xi_col = colpos[x_indices].astype(np.int32)

    # dense one-hot tiles, device layout [bin, lane, k*P+q]
    def onehots(w, q, ntiles, Kt):
        out = np.zeros((C, ntiles, P, Kt * P), BF16)
        ci, di, ki, pi = np.indices(w.shape)
        out[ci, di, pi, ki * P + q] = w.astype(BF16)
        return out

    oh12 = onehots(ohw, ohq, nt, K)
    oh3 = onehots(ohw3, ohq3, T3, K3)

    # gather index tables, device layout [lane, bin*K + k]
    gidxT = np.ascontiguousarray(gidx.transpose(0, 3, 1, 2).reshape(C, P, nt * K))
    gidxT3 = np.ascontiguousarray(gidx3.transpose(0, 3, 1, 2).reshape(C, P, T3 * K3))

    # per-core permuted x^T in [128, 4, npad] chunk layout
    F = x.shape[1]
    FC = F // P
    xT = np.zeros((C, P, FC, npad), BF16)
    for c in range(C):
        nodes = np.arange(c * nshard, (c + 1) * nshard)
        xv = x[nodes].astype(BF16)            # [nshard, F]
        for fc in range(FC):
            xT[c, :, fc, localrow[nodes]] = xv[:, fc * P:(fc + 1) * P]
    xT = xT.reshape(C, P, FC * npad)

    return dict(
        n=n, nshard=nshard, nt=nt, npad=npad, K=K, K3=K3, T3=T3, ncol=ncol,
        gidxT=gidxT, gidxT3=gidxT3, oh12=oh12, oh3=oh3, xT=xT,
        selrow_cols=selrow_cols, xi_owner=xi_owner, xi_col=xi_col,
    )


def _pack_weights(W1, b1, W2, b2, W3, b3, Wp, bp, emb, c_indices):
    """Device layouts: W [fin, fout] -> [128, nchunk*fout]; b -> [128, nchunk]."""
    def wdev(W):
        fin, fout = W.shape
        nc_ = fin // P
        return np.ascontiguousarray(
            W.astype(BF16).reshape(nc_, P, fout).transpose(1, 0, 2).reshape(P, nc_ * fout))

    def bdev(b):
        nc_ = len(b) // P
        return np.ascontiguousarray(
            np.asarray(b, np.float32).reshape(nc_, P).T)

    c_idx = np.asarray(c_indices, np.int64)
    ncg = (len(c_idx) + P - 1) // P
    tmp = np.zeros(ncg * P, np.int32)
    tmp[:len(c_idx)] = c_idx
    cidx_dev = np.ascontiguousarray(tmp.reshape(ncg, P).T)
    return dict(
        W1=wdev(W1), W2=wdev(W2), W3=wdev(W3), Wp=wdev(Wp),
        b1=bdev(b1), b2=bdev(b2), b3=bdev(b3), bp=bdev(bp),
        emb=np.asarray(emb, np.float32), cidx=cidx_dev, ncg=ncg,
    )


# ----------------------------------------------------------------------------
# bass program
# ----------------------------------------------------------------------------

def build_program(meta):
    import concourse.bacc as bacc
    import concourse.bass as bass
    import concourse.mybir as mybir
    import concourse.tile as tile
    from concourse.masks import make_identity

    nt, npad, K = meta["nt"], meta["npad"], meta["K"]
    K3, T3, ncol = meta["K3"], meta["T3"], meta["ncol"]
    ncg = meta["ncg"]
    hid, out_f = meta["hid"], meta["out_f"]
    n_cell, n_dim, n_c = meta["n_cell"], meta["n_dim"], meta["n_c"]
    nrows = C * npad
    FCH = hid // P            # chunks of hidden width
    FCO = out_f // P          # chunks of layer-3 output width
    dt = mybir.dt
    AF = mybir.ActivationFunctionType

    nc = bacc.Bacc("TRN2", target_bir_lowering=False, debug=False, num_devices=C)

    def din(name, shape, dtype):
        return nc.dram_tensor(name, list(shape), dtype, kind="ExternalInput").ap()

    xT_d = din("xT", (P, FCH * npad), dt.bfloat16)
    oh12_d = din("oh12", (nt, P, K * P), dt.bfloat16)
    oh3_d = din("oh3", (T3, P, K3 * P), dt.bfloat16)
    gidxT_d = din("gidxT", (P, nt * K), dt.int32)
    gidxT3_d = din("gidxT3", (P, T3 * K3), dt.int32)
    cidx_d = din("cidx", (P, ncg), dt.int32)
    W1_d = din("W1", (P, FCH * hid), dt.bfloat16)
    W2_d = din("W2", (P, FCH * hid), dt.bfloat16)
    W3_d = din("W3", (P, FCH * out_f), dt.bfloat16)
    Wp_d = din("Wp", (P, FCO * n_dim), dt.bfloat16)
    b1_d = din("b1", (P, FCH), dt.float32)
    b2_d = din("b2", (P, FCH), dt.float32)
    b3_d = din("b3", (P, FCO), dt.float32)
    bp_d = din("bp", (P, 1), dt.float32)
    emb_d = din("emb", (n_cell, n_dim), dt.float32)

    out_d = nc.dram_tensor("outc", [n_c, ncol], dt.float32, kind="ExternalOutput").ap()
    if meta.get("debug"):
        dbg_h3 = nc.dram_tensor("dbg_h3", [P, (out_f // P) * ncol], dt.bfloat16,
                                kind="ExternalOutput").ap()
        dbg_proj = nc.dram_tensor("dbg_proj", [P, ncol], dt.bfloat16,
                                  kind="ExternalOutput").ap()
        dbg_emb = nc.dram_tensor("dbg_emb", [P, ncg * P], dt.bfloat16,
                                 kind="ExternalOutput").ap()

    zfull = [
        nc.dram_tensor(f"zfull{i}", [nrows, hid if i < 2 else out_f], dt.bfloat16,
                       kind="Internal", addr_space="Shared").ap()
        for i in range(3)
    ]

    with tile.TileContext(nc) as tc:
        with tc.tile_pool(name="dram", bufs=1, space="DRAM") as dram, \
             tc.tile_pool(name="persist", bufs=1) as persist, \
             tc.tile_pool(name="wpool", bufs=1) as wpool, \
             tc.tile_pool(name="sbuf", bufs=3) as sbuf, \
             tc.tile_pool(name="msgp", bufs=2 * max(K, K3) + 2) as msgp, \
             tc.tile_pool(name="ohp", bufs=3) as ohp, \
             tc.tile_pool(name="zst", bufs=3) as zst, \
             tc.tile_pool(name="psum_d", bufs=2, space="PSUM") as psum_d, \
             tc.tile_pool(name="psum_a", bufs=4, space="PSUM") as psum_a:

            # persistent tiles
            HT_a = persist.tile([P, FCH * npad], dt.bfloat16, tag="HT_a")
            HT_b = persist.tile([P, FCH * npad], dt.bfloat16, tag="HT_b")
            H3T = persist.tile([P, FCO * ncol], dt.bfloat16, tag="H3T")
            gidxT_t = persist.tile([P, nt * K], dt.int32, tag="gidx")
            gidxT3_t = persist.tile([P, T3 * K3], dt.int32, tag="gidx3")
            ident = persist.tile([P, P], dt.float32, tag="ident")
            make_identity(nc, ident[:])

            nc.sync.dma_start(HT_a[:], xT_d[:])
            nc.sync.dma_start(gidxT_t[:], gidxT_d[:])
            nc.sync.dma_start(gidxT3_t[:], gidxT3_d[:])

            def dense(HT, W_ap, fout, zf_idx):
                """Z = H_own @ W -> DRAM (bf16, node-major), then AllGather."""
                Wt = wpool.tile([P, FCH * fout], dt.bfloat16, tag="W")
                nc.sync.dma_start(Wt[:], W_ap[:])
                zc = dram.tile([npad, fout], dt.bfloat16, tag=f"zc{zf_idx}")
                for i in range(nt):
                    ps = psum_d.tile([P, fout], dt.float32, space="PSUM", tag="pd")
                    for f in range(FCH):
                        nc.tensor.matmul(
                            ps[:],
                            lhsT=HT[:, f * npad + i * P: f * npad + (i + 1) * P],
                            rhs=Wt[:, f * fout:(f + 1) * fout],
                            start=(f == 0), stop=(f == FCH - 1))
                    zs = zst.tile([P, fout], dt.bfloat16, tag="zs")
                    nc.vector.tensor_copy(zs[:], ps[:])
                    nc.sync.dma_start(zc[i * P:(i + 1) * P, :], zs[:])
                nc.gpsimd.collective_compute(
                    "AllGather", mybir.AluOpType.bypass,
                    replica_groups=[list(range(C))],
                    ins=[zc[:]], outs=[zfull[zf_idx]])

            def aggregate(zf_idx, oh_ap, gidx_t, b_ap, HTout, ntiles, Kt, fch):
                """H_out^T[:, bin] = relu( sum_k msg_k^T @ oh_k + b )."""
                bt = wpool.tile([P, fch], dt.float32, tag="b")
                nc.sync.dma_start(bt[:], b_ap[:])
                zf = zfull[zf_idx]
                for d in range(ntiles):
                    oh_t = ohp.tile([P, Kt * P], dt.bfloat16, tag="oh")
                    nc.sync.dma_start(oh_t[:], oh_ap[d])
                    ps = psum_a.tile([P, fch * P], dt.float32, space="PSUM", tag="pa")
                    msgs = []
                    for k in range(Kt):
                        msg = msgp.tile([P, fch * P], dt.bfloat16, tag="msg")
                        nc.gpsimd.indirect_dma_start(
                            out=msg[:], out_offset=None, in_=zf,
                            in_offset=bass.IndirectOffsetOnAxis(
                                ap=gidx_t[:, d * Kt + k: d * Kt + k + 1], axis=0))
                        msgs.append(msg)
                    for f in range(fch):
                        for k in range(Kt):
                            nc.tensor.matmul(
                                ps[:, f * P:(f + 1) * P],
                                lhsT=msgs[k][:, f * P:(f + 1) * P],
                                rhs=oh_t[:, k * P:(k + 1) * P],
                                start=(k == 0), stop=(k == Kt - 1))
                    for f in range(fch):
                        nc.scalar.activation(
                            HTout[:, f * (ntiles * P) + d * P:
                                  f * (ntiles * P) + (d + 1) * P],
                            ps[:, f * P:(f + 1) * P],
                            AF.Relu, bias=bt[:, f:f + 1])

            # ---- layers ----
            dense(HT_a, W1_d, hid, 0)
            aggregate(0, oh12_d, gidxT_t, b1_d, HT_b, nt, K, FCH)
            dense(HT_b, W2_d, hid, 1)
            aggregate(1, oh12_d, gidxT_t, b2_d, HT_a, nt, K, FCH)
            dense(HT_a, W3_d, out_f, 2)
            aggregate(2, oh3_d, gidxT3_t, b3_d, H3T, T3, K3, FCO)

            # ---- projection: projT = Wp^T @ enc^T + bp  [n_dim, ncol] ----
            Wpt = wpool.tile([P, FCO * n_dim], dt.bfloat16, tag="W")
            bpt = wpool.tile([P, 1], dt.float32, tag="b")
            nc.sync.dma_start(Wpt[:], Wp_d[:])
            nc.sync.dma_start(bpt[:], bp_d[:])
            projT = persist.tile([P, ncol], dt.bfloat16, tag="projT")
            nseg = (ncol + 511) // 512
            for s in range(nseg):
                w = min(512, ncol - s * 512)
                pp = psum_d.tile([P, 512], dt.float32, space="PSUM", tag="pd")
                for f in range(FCO):
                    nc.tensor.matmul(
                        pp[:, :w],
                        lhsT=Wpt[:, f * n_dim:(f + 1) * n_dim],
                        rhs=H3T[:, f * ncol + s * 512: f * ncol + s * 512 + w],
                        start=(f == 0), stop=(f == FCO - 1))
                nc.scalar.activation(projT[:, s * 512:s * 512 + w], pp[:, :w],
                                     AF.Identity, bias=bpt[:, 0:1])

            # ---- EmbSel^T: gather emb[c_indices] and transpose ----
            cidx_t = sbuf.tile([P, ncg], dt.int32, tag="cidx")
            nc.sync.dma_start(cidx_t[:], cidx_d[:])
            embT = persist.tile([P, ncg * P], dt.bfloat16, tag="embT")
            for g in range(ncg):
                e_g = sbuf.tile([P, n_dim], dt.float32, tag="eg")
                nc.gpsimd.indirect_dma_start(
                    out=e_g[:], out_offset=None, in_=emb_d,
                    in_offset=bass.IndirectOffsetOnAxis(
                        ap=cidx_t[:, g:g + 1], axis=0))
                pt = psum_d.tile([P, P], dt.float32, space="PSUM", tag="pd")
                nc.tensor.transpose(pt[:], e_g[:], ident[:])
                nc.vector.tensor_copy(embT[:, g * P:(g + 1) * P], pt[:])

            if meta.get("debug"):
                nc.sync.dma_start(dbg_h3[:], H3T[:])
                nc.sync.dma_start(dbg_proj[:], projT[:])
                nc.sync.dma_start(dbg_emb[:], embT[:])

            # ---- out_c = EmbSel @ projT  [N_C, ncol] ----
            for g in range(ncg):
                for s in range(nseg):
                    w = min(512, ncol - s * 512)
                    po = psum_d.tile([P, 512], dt.float32, space="PSUM", tag="pd")
                    nc.tensor.matmul(
                        po[:, :w],
                        lhsT=embT[:, g * P:(g + 1) * P],
                        rhs=projT[:, s * 512:s * 512 + w],
                        start=True, stop=True)
                    os_ = zst.tile([P, 512], dt.float32, tag="os")
                    nc.vector.tensor_copy(os_[:, :w], po[:, :w])
                    nc.sync.dma_start(
                        out_d[g * P:(g + 1) * P, s * 512:s * 512 + w],
                        os_[:, :w])

    nc.compile()
    return nc


# ----------------------------------------------------------------------------
# entry point
# ----------------------------------------------------------------------------

def _ensure_ntff_hook():
    """Register the axon NTFF-profile hook if the image's antenv lacks it.
    Only used on the TRACE path (benchmarking); grading runs trace=False."""
    import sys
    import types
    try:
        from antenv.axon_hooks import get_axon_ntff_profile_hook  # noqa: F401
        return
    except ImportError:
        pass
    try:
        from trn_agent_boot.trn_boot import _ntff_profile_via_ctypes
        hook = _ntff_profile_via_ctypes("/opt/axon/libaxon_pjrt.so")
    except Exception:
        hook = None
    mod = types.ModuleType("antenv.axon_hooks")
    mod._hook = hook
    mod.get_axon_ntff_profile_hook = lambda: mod._hook
    mod.set_axon_ntff_profile_hook = lambda h: setattr(mod, "_hook", h)
    import antenv
    antenv.axon_hooks = mod
    sys.modules["antenv.axon_hooks"] = mod


def kernel(**inputs):
    global LAST_EXEC_TIME_NS
    from concourse import bass_utils
    if TRACE:
        _ensure_ntff_hook()

    x = np.asarray(inputs["x"], np.float32)
    prep = preprocess(x, inputs["src"], inputs["dst"],
                      inputs["x_indices"], inputs["c_indices"])
    wp = _pack_weights(inputs["W1"], inputs["b1"], inputs["W2"], inputs["b2"],
                       inputs["W3"], inputs["b3"], inputs["Wp"], inputs["bp"],
                       inputs["emb"], inputs["c_indices"])

    hid = np.asarray(inputs["W1"]).shape[1]
    out_f = np.asarray(inputs["W3"]).shape[1]
    n_dim = np.asarray(inputs["Wp"]).shape[1]
    n_cell = np.asarray(inputs["emb"]).shape[0]
    n_c = len(np.asarray(inputs["c_indices"]))
    meta = dict(nt=prep["nt"], npad=prep["npad"], K=prep["K"], K3=prep["K3"],
                T3=prep["T3"], ncol=prep["ncol"], ncg=wp["ncg"],
                hid=hid, out_f=out_f, n_dim=n_dim, n_cell=n_cell, n_c=n_c)
    meta_key = tuple(sorted(meta.items()))
    if meta_key not in _COMPILE_CACHE:
        _COMPILE_CACHE[meta_key] = build_program(meta)
    nc = _COMPILE_CACHE[meta_key]

    in_maps = []
    for c in range(C):
        in_maps.append({
            "xT": prep["xT"][c],
            "oh12": prep["oh12"][c],
            "oh3": prep["oh3"][c],
            "gidxT": prep["gidxT"][c],
            "gidxT3": prep["gidxT3"][c],
            "cidx": wp["cidx"],
            "W1": wp["W1"], "W2": wp["W2"], "W3": wp["W3"], "Wp": wp["Wp"],
            "b1": wp["b1"], "b2": wp["b2"], "b3": wp["b3"], "bp": wp["bp"],
            "emb": wp["emb"],
        })

    res = bass_utils.run_bass_kernel_spmd(
        nc, in_maps, core_ids=list(range(C)), trace=TRACE)
    LAST_EXEC_TIME_NS = res.exec_time_ns
    globals()["LAST_RESULTS"] = res

    outs = np.stack([r["outc"] for r in res.results])     # [C, N_C, ncol]
    final = outs[prep["xi_owner"], :, prep["xi_col"]]     # [N_SEL, N_C]
    return np.ascontiguousarray(final.T, np.float32)      # [N_C, N_SEL]


# revision 35
# speedup vs baseline: 1.2254x; 1.2213x over previous
"""Trainium2 Bass kernel for Cell2Vec GNN message passing (8 NeuronCores).

Math: 3x GraphConv (DGL norm='both') + node-select + projection + cell-embedding
scores:
    out = emb[c_indices] @ (relu-chain...)  -> [N_C, N_SEL]

Restructure used on device (per layer):
    H_next = relu( Ahat @ (H @ W) + b ),  Ahat = D_in^-1/2 A D_out^-1/2
with the degree norms folded into per-edge weights w_e = ns[src] * nd[dst].

Sharding: nodes are dst-sharded across 8 cores (6250 each, padded 6272 = 49
tiles of 128). Per layer, each core computes Z = H_own @ W (dense, PE), an
AllGather replicates Z to all cores, then each core aggregates its owned
dst-nodes: for each dst-bin (128 nodes) and edge-tile (128 edges), gather the
128 src rows of Z (indirect DMA) and accumulate on the tensor engine
    aggT[feat, dstslot] += msg[lane, feat]^T @ Onehot[lane, dstslot]
where Onehot carries w_e at (lane, dst_slot). This yields H_next^T directly
(feature-major), which is exactly the lhsT layout the next dense needs.
Layer 3 only aggregates into the x_indices-selected nodes. The final
projection + emb @ proj^T runs per-core on owned selected columns; the host
reassembles the [1024, 8192] output from per-core column blocks.

Bins are in-degree balanced per core (host preprocessing) so every bin has
the same number of edge tiles K; all 8 cores run one identical SPMD program.
"""
import heapq
import numpy as np
import ml_dtypes

P = 128
C = 8

# full-problem config (hardcoded per spec; kernel.py must be self-contained)
N_NODES = 50000
N_EDGES = 400000
IN_F = 512
HID = 512
OUT_F = 256
N_CELL = 1000
N_DIM = 128
N_SEL = 8192
N_C = 1024

BF16 = ml_dtypes.bfloat16

_COMPILE_CACHE = {}
LAST_EXEC_TIME_NS = None
TRACE = False


# ----------------------------------------------------------------------------
# host preprocessing
# ----------------------------------------------------------------------------

def _balance_bins(weights, n_bins, cap):
    """Greedy balanced binning: heaviest first into least-loaded open bin.
    Returns (bin_of_item, slot_of_item)."""
    order = np.argsort(-weights, kind="stable")
    heap = [(0.0, b) for b in range(n_bins)]
    heapq.heapify(heap)
    counts = np.zeros(n_bins, np.int64)
    bin_of = np.empty(len(weights), np.int64)
    slot_of = np.empty(len(weights), np.int64)
    for i in order:
        spill = []
        while True:
            load, b = heapq.heappop(heap)
            if counts[b] < cap:
                break
            spill.append((load, b))
        bin_of[i] = b
        slot_of[i] = counts[b]
        counts[b] += 1
        heapq.heappush(heap, (load + float(weights[i]), b))
        for s in spill:
            heapq.heappush(heap, s)
    return bin_of, slot_of


def _group_edges(key, n_groups, payload_order):
    """Sort edges by group key; return per-group start/end and sorted order."""
    order = np.argsort(key, kind="stable")
    ks = key[order]
    gs = np.searchsorted(ks, np.arange(n_groups))
    ge = np.searchsorted(ks, np.arange(n_groups), side="right")
    return order, ks, gs, ge


def _build_graph_tables(isB, relrow, w_e, qslot, group_c, group_d, C_, ntiles):
    """Per-(core, bin) edge layout for batched dma_gather.

    Edges of each bin are split into two gather tables (A: first row-chunk of
    every shard, B: second — matches the chunked AllGather), laid out
    A-tiles-then-B-tiles, padded to uniform global (Klo, Khi). Returns:
      Klo, Khi,
      oh   [C, ntiles, P, (Klo+Khi)*P]  bf16 one-hot (w at (lane, k*P+q)),
      idxw [C, P, ntiles*(Klo+Khi)*8]   int16 wrapped gather indices
           (per bin: Klo*8 A-columns then Khi*8 B-columns).
    """
    E = len(relrow)
    hi = np.asarray(isB).astype(np.int64)
    key = (group_c * ntiles + group_d) * 2 + hi
    order = np.argsort(key, kind="stable")
    ks = key[order]
    ngroups = C_ * ntiles * 2
    gs = np.searchsorted(ks, np.arange(ngroups))
    ge = np.searchsorted(ks, np.arange(ngroups), side="right")
    cnt = (ge - gs).reshape(C_, ntiles, 2)
    Klo = max(1, int(np.ceil(cnt[:, :, 0].max() / P)))
    Khi = max(1, int(np.ceil(cnt[:, :, 1].max() / P)))
    K = Klo + Khi

    pos = np.arange(E) - gs[ks]                  # position within (c,d,half)
    cc = ks // (2 * ntiles)
    dd = (ks // 2) % ntiles
    hh = ks % 2
    tile_ = np.where(hh == 0, pos // P, Klo + pos // P)
    lane = pos % P
    es = order

    oh = np.zeros((C_, ntiles, P, K * P), BF16)
    oh[cc, dd, lane, tile_ * P + qslot[es]] = w_e[es].astype(BF16)

    # relative int16 indices, padded slots point at row 0 (weight 0)
    ilo = np.zeros((C_, ntiles, Klo * P), np.int16)
    ihi = np.zeros((C_, ntiles, Khi * P), np.int16)
    mlo, mhi = hh == 0, hh == 1
    ilo[cc[mlo], dd[mlo], pos[mlo]] = relrow[es[mlo]].astype(np.int16)
    ihi[cc[mhi], dd[mhi], pos[mhi]] = relrow[es[mhi]].astype(np.int16)

    def wrap(v):   # [..., L] -> [..., 16, L//16] with unwrapped[j] = w[j%16, j//16]
        shp = v.shape[:-1]
        L = v.shape[-1]
        return v.reshape(*shp, L // 16, 16).swapaxes(-1, -2)

    wlo = wrap(ilo)                               # [C, ntiles, 16, Klo*8]
    whi = wrap(ihi)
    percol = np.concatenate([wlo, whi], axis=-1)  # [C, ntiles, 16, K*8]
    percol = percol.transpose(0, 2, 1, 3).reshape(C_, 16, ntiles * K * 8)
    idxw = np.tile(percol, (1, P // 16, 1))       # replicate to 128 partitions
    return Klo, Khi, np.ascontiguousarray(oh), np.ascontiguousarray(idxw)


def preprocess(x, src, dst, x_indices, c_indices):
    src = np.asarray(src).astype(np.int64)
    dst = np.asarray(dst).astype(np.int64)
    x_indices = np.asarray(x_indices).astype(np.int64)
    c_indices = np.asarray(c_indices).astype(np.int64)
    x = np.asarray(x)
    n = x.shape[0]
    nshard = n // C
    nt = (nshard + P - 1) // P
    npad = nt * P
    nt_a = (nt + 1) // 2           # AllGather chunk A = first nt_a tiles
    rows_a, rows_b = nt_a * P, (nt - nt_a) * P

    deg_out = np.bincount(src, minlength=n).astype(np.float64)
    deg_in = np.bincount(dst, minlength=n).astype(np.float64)
    ns = np.where(deg_out > 0, 1.0 / np.sqrt(np.maximum(deg_out, 1.0)), 0.0)
    nd = np.where(deg_in > 0, 1.0 / np.sqrt(np.maximum(deg_in, 1.0)), 0.0)
    w_e = (ns[src] * nd[dst]).astype(np.float32)

    owner_n = np.arange(n) // nshard
    localrow = np.empty(n, np.int64)
    for c in range(C):
        nodes = np.arange(c * nshard, (c + 1) * nshard)
        b, s = _balance_bins(deg_in[nodes], nt, P)
        localrow[nodes] = b * P + s
    # chunked-AllGather relative row: table A holds rows [0, rows_a) of every
    # shard (concatenated by owner), table B the rest.
    isB_n = localrow >= rows_a
    relrow_n = np.where(isB_n, owner_n * rows_b + (localrow - rows_a),
                        owner_n * rows_a + localrow)

    # L1/L2 edge layout
    Klo, Khi, oh12, idxw12 = _build_graph_tables(
        isB_n[src], relrow_n[src], w_e, (localrow[dst] % P).astype(np.int64),
        dst // nshard, localrow[dst] // P, C, nt)

    # L3: selected nodes only
    sel_nodes = np.unique(x_indices)
    sel_mask = np.zeros(n, bool)
    sel_mask[sel_nodes] = True
    e3 = np.nonzero(sel_mask[dst])[0]
    deg3 = np.bincount(dst[e3], minlength=n).astype(np.float64)
    ncol_max = max(int((sel_nodes // nshard == c).sum()) for c in range(C))
    T3 = max(1, (ncol_max + P - 1) // P)
    ncol = T3 * P
    colpos = np.full(n, 0, np.int64)
    for c in range(C):
        nodes = sel_nodes[sel_nodes // nshard == c]
        b, s = _balance_bins(deg3[nodes], T3, P)
        colpos[nodes] = b * P + s
    K3lo, K3hi, oh3, idxw3 = _build_graph_tables(
        isB_n[src[e3]], relrow_n[src[e3]], w_e[e3],
        (colpos[dst[e3]] % P).astype(np.int64),
        dst[e3] // nshard, colpos[dst[e3]] // P, C, T3)

    xi_owner = (x_indices // nshard).astype(np.int32)
    xi_col = colpos[x_indices].astype(np.int32)

    # per-core permuted x^T in [128, 4, npad] chunk layout
    F = x.shape[1]
    FC = F // P
    xT = np.zeros((C, P, FC, npad), BF16)
    for c in range(C):
        nodes = np.arange(c * nshard, (c + 1) * nshard)
        xv = x[nodes].astype(BF16)            # [nshard, F]
        for fc in range(FC):
            xT[c, :, fc, localrow[nodes]] = xv[:, fc * P:(fc + 1) * P]
    xT = xT.reshape(C, P, FC * npad)

    return dict(
        n=n, nshard=nshard, nt=nt, npad=npad, T3=T3, ncol=ncol,
        Klo=Klo, Khi=Khi, K3lo=K3lo, K3hi=K3hi,
        idxw12=idxw12, idxw3=idxw3, oh12=oh12, oh3=oh3, xT=xT,
        xi_owner=xi_owner, xi_col=xi_col,
    )


def _pack_weights(W1, b1, W2, b2, W3, b3, Wp, bp, emb, c_indices):
    """Device layouts: W [fin, fout] -> [128, nchunk*fout]; b -> [128, nchunk]."""
    def wdev(W):
        fin, fout = W.shape
        nc_ = fin // P
        return np.ascontiguousarray(
            W.astype(BF16).reshape(nc_, P, fout).transpose(1, 0, 2).reshape(P, nc_ * fout))

    def bdev(b):
        nc_ = len(b) // P
        return np.ascontiguousarray(
            np.asarray(b, np.float32).reshape(nc_, P).T)

    c_idx = np.asarray(c_indices, np.int64)
    ncg = (len(c_idx) + P - 1) // P
    tmp = np.zeros(ncg * P, np.int16)
    tmp[:len(c_idx)] = c_idx
    # wrapped int16 for dma_gather: idx j at [j % 16, j // 16], replicated x8
    cidx_dev = np.ascontiguousarray(
        np.tile(tmp.reshape(ncg * 8, 16).T, (P // 16, 1)))
    return dict(
        W1=wdev(W1), W2=wdev(W2), W3=wdev(W3), Wp=wdev(Wp),
        b1=bdev(b1), b2=bdev(b2), b3=bdev(b3), bp=bdev(bp),
        emb=np.asarray(emb, np.float32), cidx=cidx_dev, ncg=ncg,
    )


# ----------------------------------------------------------------------------
# bass program
# ----------------------------------------------------------------------------

def build_program(meta):
    import concourse.bacc as bacc
    import concourse.bass as bass
    import concourse.mybir as mybir
    import concourse.tile as tile
    from concourse.masks import make_identity

    nt, npad = meta["nt"], meta["npad"]
    T3, ncol = meta["T3"], meta["ncol"]
    Klo, Khi = meta["Klo"], meta["Khi"]
    K3lo, K3hi = meta["K3lo"], meta["K3hi"]
    K = Klo + Khi
    K3 = K3lo + K3hi
    ncg = meta["ncg"]
    hid, out_f = meta["hid"], meta["out_f"]
    n_cell, n_dim, n_c = meta["n_cell"], meta["n_dim"], meta["n_c"]
    nt_a = (nt + 1) // 2
    nt_b = nt - nt_a
    rows_a, rows_b = nt_a * P, nt_b * P
    FCH = hid // P            # chunks of hidden width
    FCO = out_f // P          # chunks of layer-3 output width
    dt = mybir.dt
    AF = mybir.ActivationFunctionType

    nc = bacc.Bacc("TRN2", target_bir_lowering=False, debug=False, num_devices=C,
                   num_swdge_queues=4)

    def din(name, shape, dtype):
        return nc.dram_tensor(name, list(shape), dtype, kind="ExternalInput").ap()

    xT_d = din("xT", (P, FCH * npad), dt.bfloat16)
    oh12_d = din("oh12", (nt, P, K * P), dt.bfloat16)
    oh3_d = din("oh3", (T3, P, K3 * P), dt.bfloat16)
    idxw12_d = din("idxw12", (P, nt * K * 8), dt.int16)
    idxw3_d = din("idxw3", (P, T3 * K3 * 8), dt.int16)
    cidx_d = din("cidx", (P, ncg * 8), dt.int16)
    W1_d = din("W1", (P, FCH * hid), dt.bfloat16)
    W2_d = din("W2", (P, FCH * hid), dt.bfloat16)
    W3_d = din("W3", (P, FCH * out_f), dt.bfloat16)
    Wp_d = din("Wp", (P, FCO * n_dim), dt.bfloat16)
    b1_d = din("b1", (P, FCH), dt.float32)
    b2_d = din("b2", (P, FCH), dt.float32)
    b3_d = din("b3", (P, FCO), dt.float32)
    bp_d = din("bp", (P, 1), dt.float32)
    emb_d = din("emb", (n_cell, n_dim), dt.float32)

    out_d = nc.dram_tensor("outc", [n_c, ncol], dt.float32, kind="ExternalOutput").ap()
    if meta.get("debug"):
        dbg_h3 = nc.dram_tensor("dbg_h3", [P, (out_f // P) * ncol], dt.bfloat16,
                                kind="ExternalOutput").ap()
        dbg_proj = nc.dram_tensor("dbg_proj", [P, ncol], dt.bfloat16,
                                  kind="ExternalOutput").ap()
        dbg_emb = nc.dram_tensor("dbg_emb", [P, ncg * P], dt.bfloat16,
                                 kind="ExternalOutput").ap()

    zfull = [
        (nc.dram_tensor(f"zfullA{i}", [C * rows_a, hid if i < 2 else out_f],
                        dt.bfloat16, kind="Internal", addr_space="Shared").ap(),
         nc.dram_tensor(f"zfullB{i}", [C * rows_b, hid if i < 2 else out_f],
                        dt.bfloat16, kind="Internal", addr_space="Shared").ap())
        for i in range(3)
    ]

    from concourse import library_config

    with tile.TileContext(nc) as tc:
        with tc.tile_pool(name="dram", bufs=1, space="DRAM") as dram, \
             tc.tile_pool(name="persist", bufs=1) as persist, \
             tc.tile_pool(name="wpool", bufs=1) as wpool, \
             tc.tile_pool(name="sbuf", bufs=3) as sbuf, \
             tc.tile_pool(name="msgp", bufs=3) as msgp, \
             tc.tile_pool(name="ohp", bufs=3) as ohp, \
             tc.tile_pool(name="zst", bufs=3) as zst, \
             tc.tile_pool(name="psum_d", bufs=2, space="PSUM") as psum_d, \
             tc.tile_pool(name="psum_a", bufs=4, space="PSUM") as psum_a:

            nc.gpsimd.load_library(library_config.mlp)
            gq = [0]          # global SWDGE queue rotation (lane i <-> queue i%4)

            def next_q():
                q = gq[0] % 4
                gq[0] += 1
                return q

            # persistent tiles
            HT_a = persist.tile([P, FCH * npad], dt.bfloat16, tag="HT_a")
            HT_b = persist.tile([P, FCH * npad], dt.bfloat16, tag="HT_b")
            H3T = persist.tile([P, FCO * ncol], dt.bfloat16, tag="H3T")
            idxw12_t = persist.tile([P, nt * K * 8], dt.int16, tag="gidx")
            idxw3_t = persist.tile([P, T3 * K3 * 8], dt.int16, tag="gidx3")
            ident = persist.tile([P, P], dt.float32, tag="ident")
            make_identity(nc, ident[:])

            nc.sync.dma_start(HT_a[:], xT_d[:])
            nc.sync.dma_start(idxw12_t[:], idxw12_d[:])
            nc.sync.dma_start(idxw3_t[:], idxw3_d[:])

            def dense(HT, W_ap, fout, zf_idx):
                """Z = H_own @ W -> DRAM (bf16, node-major), AllGather in two
                row-chunks so aggregation can start after chunk A lands."""
                Wt = wpool.tile([P, FCH * fout], dt.bfloat16, tag="W")
                nc.sync.dma_start(Wt[:], W_ap[:])
                zca = dram.tile([rows_a, fout], dt.bfloat16, tag=f"zca{zf_idx}")
                zcb = dram.tile([rows_b, fout], dt.bfloat16, tag=f"zcb{zf_idx}")
                for i in range(nt):
                    ps = psum_d.tile([P, fout], dt.float32, space="PSUM", tag="pd")
                    for f in range(FCH):
                        nc.tensor.matmul(
                            ps[:],
                            lhsT=HT[:, f * npad + i * P: f * npad + (i + 1) * P],
                            rhs=Wt[:, f * fout:(f + 1) * fout],
                            start=(f == 0), stop=(f == FCH - 1))
                    zs = zst.tile([P, fout], dt.bfloat16, tag="zs")
                    nc.vector.tensor_copy(zs[:], ps[:])
                    if i < nt_a:
                        nc.sync.dma_start(zca[i * P:(i + 1) * P, :], zs[:])
                    else:
                        j = i - nt_a
                        nc.sync.dma_start(zcb[j * P:(j + 1) * P, :], zs[:])
                    if i == nt_a - 1:
                        nc.gpsimd.collective_compute(
                            "AllGather", mybir.AluOpType.bypass,
                            replica_groups=[list(range(C))],
                            ins=[zca[:]], outs=[zfull[zf_idx][0]])
                nc.gpsimd.collective_compute(
                    "AllGather", mybir.AluOpType.bypass,
                    replica_groups=[list(range(C))],
                    ins=[zcb[:]], outs=[zfull[zf_idx][1]])

            def aggregate(zf_idx, oh_ap, idx_t, b_ap, HTout, ntiles, klo, khi, fch):
                """H_out^T[:, bin] = relu( sum_k msg_k^T @ oh_k + b ).
                Per bin: two batched dma_gathers (lo/hi half of zfull)."""
                kt = klo + khi
                elem = fch * P
                bt = wpool.tile([P, fch], dt.float32, tag="b")
                nc.sync.dma_start(bt[:], b_ap[:])
                zfa, zfb = zfull[zf_idx]
                for d in range(ntiles):
                    oh_t = ohp.tile([P, kt * P], dt.bfloat16, tag="oh")
                    nc.sync.dma_start(oh_t[:], oh_ap[d])
                    ps = psum_a.tile([P, fch * P], dt.float32, space="PSUM", tag="pa")
                    msg = msgp.tile([P, kt, elem], dt.bfloat16, tag="msg")
                    icol = d * kt * 8
                    nc.gpsimd.dma_gather(
                        msg[:, 0:klo, :], zfa[:],
                        idx_t[:, icol: icol + klo * 8],
                        klo * P, klo * P, elem, queue_num=next_q())
                    nc.gpsimd.dma_gather(
                        msg[:, klo:kt, :], zfb[:],
                        idx_t[:, icol + klo * 8: icol + kt * 8],
                        khi * P, khi * P, elem, queue_num=next_q())
                    for f in range(fch):
                        for k in range(kt):
                            nc.tensor.matmul(
                                ps[:, f * P:(f + 1) * P],
                                lhsT=msg[:, k, f * P:(f + 1) * P],
                                rhs=oh_t[:, k * P:(k + 1) * P],
                                start=(k == 0), stop=(k == kt - 1))
                    for f in range(fch):
                        nc.scalar.activation(
                            HTout[:, f * (ntiles * P) + d * P:
                                  f * (ntiles * P) + (d + 1) * P],
                            ps[:, f * P:(f + 1) * P],
                            AF.Relu, bias=bt[:, f:f + 1])

            # ---- layers ----
            dense(HT_a, W1_d, hid, 0)
            aggregate(0, oh12_d, idxw12_t, b1_d, HT_b, nt, Klo, Khi, FCH)
            dense(HT_b, W2_d, hid, 1)
            aggregate(1, oh12_d, idxw12_t, b2_d, HT_a, nt, Klo, Khi, FCH)
            dense(HT_a, W3_d, out_f, 2)
            aggregate(2, oh3_d, idxw3_t, b3_d, H3T, T3, K3lo, K3hi, FCO)

            # ---- projection: projT = Wp^T @ enc^T + bp  [n_dim, ncol] ----
            Wpt = wpool.tile([P, FCO * n_dim], dt.bfloat16, tag="W")
            bpt = wpool.tile([P, 1], dt.float32, tag="b")
            nc.sync.dma_start(Wpt[:], Wp_d[:])
            nc.sync.dma_start(bpt[:], bp_d[:])
            projT = persist.tile([P, ncol], dt.bfloat16, tag="projT")
            nseg = (ncol + 511) // 512
            for s in range(nseg):
                w = min(512, ncol - s * 512)
                pp = psum_d.tile([P, 512], dt.float32, space="PSUM", tag="pd")
                for f in range(FCO):
                    nc.tensor.matmul(
                        pp[:, :w],
                        lhsT=Wpt[:, f * n_dim:(f + 1) * n_dim],
                        rhs=H3T[:, f * ncol + s * 512: f * ncol + s * 512 + w],
                        start=(f == 0), stop=(f == FCO - 1))
                nc.scalar.activation(projT[:, s * 512:s * 512 + w], pp[:, :w],
                                     AF.Identity, bias=bpt[:, 0:1])

            # ---- EmbSel^T: gather emb[c_indices] and transpose ----
            cidx_t = sbuf.tile([P, ncg * 8], dt.int16, tag="cidx")
            nc.sync.dma_start(cidx_t[:], cidx_d[:])
            embT = persist.tile([P, ncg * P], dt.bfloat16, tag="embT")
            e_all = sbuf.tile([P, ncg, n_dim], dt.float32, tag="eg")
            nc.gpsimd.dma_gather(
                e_all[:], emb_d[:], cidx_t[:], ncg * P, ncg * P, n_dim,
                queue_num=next_q())
            for g in range(ncg):
                pt = psum_d.tile([P, P], dt.float32, space="PSUM", tag="pd")
                nc.tensor.transpose(pt[:], e_all[:, g, :], ident[:])
                nc.vector.tensor_copy(embT[:, g * P:(g + 1) * P], pt[:])

            if meta.get("debug"):
                nc.sync.dma_start(dbg_h3[:], H3T[:])
                nc.sync.dma_start(dbg_proj[:], projT[:])
                nc.sync.dma_start(dbg_emb[:], embT[:])

            # ---- out_c = EmbSel @ projT  [N_C, ncol] ----
            for g in range(ncg):
                for s in range(nseg):
                    w = min(512, ncol - s * 512)
                    po = psum_d.tile([P, 512], dt.float32, space="PSUM", tag="pd")
                    nc.tensor.matmul(
                        po[:, :w],
                        lhsT=embT[:, g * P:(g + 1) * P],
                        rhs=projT[:, s * 512:s * 512 + w],
                        start=True, stop=True)
                    os_ = zst.tile([P, 512], dt.float32, tag="os")
                    nc.vector.tensor_copy(os_[:, :w], po[:, :w])
                    nc.sync.dma_start(
                        out_d[g * P:(g + 1) * P, s * 512:s * 512 + w],
                        os_[:, :w])

    nc.compile()
    return nc


# ----------------------------------------------------------------------------
# entry point
# ----------------------------------------------------------------------------

def _ensure_ntff_hook():
    """Register the axon NTFF-profile hook if the image's antenv lacks it.
    Only used on the TRACE path (benchmarking); grading runs trace=False."""
    import sys
    import types
    try:
        from antenv.axon_hooks import get_axon_ntff_profile_hook  # noqa: F401
        return
    except ImportError:
        pass
    try:
        from trn_agent_boot.trn_boot import _ntff_profile_via_ctypes
        hook = _ntff_profile_via_ctypes("/opt/axon/libaxon_pjrt.so")
    except Exception:
        hook = None
    mod = types.ModuleType("antenv.axon_hooks")
    mod._hook = hook
    mod.get_axon_ntff_profile_hook = lambda: mod._hook
    mod.set_axon_ntff_profile_hook = lambda h: setattr(mod, "_hook", h)
    import antenv
    antenv.axon_hooks = mod
    sys.modules["antenv.axon_hooks"] = mod


def kernel(**inputs):
    global LAST_EXEC_TIME_NS
    from concourse import bass_utils
    if TRACE:
        _ensure_ntff_hook()

    x = np.asarray(inputs["x"], np.float32)
    prep = preprocess(x, inputs["src"], inputs["dst"],
                      inputs["x_indices"], inputs["c_indices"])
    wp = _pack_weights(inputs["W1"], inputs["b1"], inputs["W2"], inputs["b2"],
                       inputs["W3"], inputs["b3"], inputs["Wp"], inputs["bp"],
                       inputs["emb"], inputs["c_indices"])

    hid = np.asarray(inputs["W1"]).shape[1]
    out_f = np.asarray(inputs["W3"]).shape[1]
    n_dim = np.asarray(inputs["Wp"]).shape[1]
    n_cell = np.asarray(inputs["emb"]).shape[0]
    n_c = len(np.asarray(inputs["c_indices"]))
    meta = dict(nt=prep["nt"], npad=prep["npad"],
                Klo=prep["Klo"], Khi=prep["Khi"],
                K3lo=prep["K3lo"], K3hi=prep["K3hi"],
                T3=prep["T3"], ncol=prep["ncol"], ncg=wp["ncg"],
                hid=hid, out_f=out_f, n_dim=n_dim, n_cell=n_cell, n_c=n_c)
    meta_key = tuple(sorted(meta.items()))
    if meta_key not in _COMPILE_CACHE:
        _COMPILE_CACHE[meta_key] = build_program(meta)
    nc = _COMPILE_CACHE[meta_key]

    in_maps = []
    for c in range(C):
        in_maps.append({
            "xT": prep["xT"][c],
            "oh12": prep["oh12"][c],
            "oh3": prep["oh3"][c],
            "idxw12": prep["idxw12"][c],
            "idxw3": prep["idxw3"][c],
            "cidx": wp["cidx"],
            "W1": wp["W1"], "W2": wp["W2"], "W3": wp["W3"], "Wp": wp["Wp"],
            "b1": wp["b1"], "b2": wp["b2"], "b3": wp["b3"], "bp": wp["bp"],
            "emb": wp["emb"],
        })

    res = bass_utils.run_bass_kernel_spmd(
        nc, in_maps, core_ids=list(range(C)), trace=TRACE)
    LAST_EXEC_TIME_NS = res.exec_time_ns
    globals()["LAST_RESULTS"] = res

    outs = np.stack([r["outc"] for r in res.results])     # [C, N_C, ncol]
    final = outs[prep["xi_owner"], :, prep["xi_col"]]     # [N_SEL, N_C]
    return np.ascontiguousarray(final.T, np.float32)      # [N_C, N_SEL]
